# revision 6
# baseline (speedup 1.0000x reference)
"""Trainium2 Bass kernel for single-head attention (no V projection).

Reference computation (per batch b):
    qk   = x @ W_qk.T + b_qk          # [n, 2d]
    q, k = qk[:, :d], qk[:, d:]
    dots[i, j] = k_i . q_j / sqrt(d)
    attn = softmax(dots, axis=-1)
    out[i] = sum_j attn[i, j] * x[j]

Because there is no V projection, the q/k projections fold into a single
matrix: dots = x @ M @ x.T / sqrt(d) with M = Wk.T @ Wq precomputed on host
([d, d]).  That replaces the [n,d]x[d,2d] projection (8.6 GF) with
y = x @ M (4.3 GF) and keeps y on-chip (no DRAM spill), cutting tensor-engine
work from 25.8 GF to 21.5 GF per core.  The bias enters softmax only through
a per-key offset v_j = x_j . (Wq.T @ bk) (row-constant terms cancel), folded
into the exp as a per-partition ACT bias.

Sharding: data-parallel over batch b (8 batches -> 8 NeuronCores), no
collectives.  Per core:

  A:  yT[m,:] = M^T-chunks^T @ xT      (PE, fp16 operands, fp32 PSUM)
  B:  sT[j,i] = xT-chunk^T @ yT-chunk; E^T = exp(sT/32 + v_j)  (ACT)
  C:  out[i,:] = E^T(:,i)^T @ X in PSUM over j; softmax denominator from a
      DVE running sum of E^T strips + one tiny ones-matmul per 128-row block;
      normalize via per-partition reciprocal (DVE).

exp() is computed without max-subtraction: scores are ~N(0, 0.67) after the
1/sqrt(d) scale, so exp never overflows (max |s/32| ~ 5.4 on this data) and
softmax(x) == exp(x)/sum(exp(x)).

All matmul operands are fp16 (full PE rate; rel err ~6e-4 vs fp32 reference).

Host-side input layouts (computed in kernel()):
  m4 [128, 8*8*128]: m4[p, ((m*8)+k)*128+c] = M[k*128+p, m*128+c]
  xt [128, 8*2048]:  xt[p, k*2048+i] = x[i, k*128+p]
  xn [2048, 1024]:   x in fp16
  vb [128, 16]:      vb[p, j] = (x @ (Wq.T@bk))[j*128+p] / 32
"""
import sys

try:
    import concourse.bass as bass  # noqa: F401
except ImportError:  # pragma: no cover
    sys.path.insert(0, "/opt/trn_rl_repo")

import numpy as np
import concourse.bass as bass
import concourse.mybir as mybir
import concourse.tile as tile
from concourse import bacc
from concourse.bass_utils import run_bass_kernel_spmd
import concourse.bass_utils as _bu

# NOTE: unlike the fp32r version, we do NOT patch walrus to
# --enable-ldw-opt=true: fp16 weights take the Fast-Weight-Load path, which
# that optimization rejects (InstLdweights incompatible), and FWL's 2x faster
# weight loads matter more than eliding the duplicate LDWEIGHTS in stage C.

B, N, D = 8, 2048, 1024
NCORES = 8
SCALE = 1.0 / np.sqrt(D)  # 1/32

_NC = None
LAST_RESULTS = None


def _build_nc():
    H = mybir.dt.float16
    F = mybir.dt.float32
    nc = bacc.Bacc("TRN2", target_bir_lowering=False, debug=False, num_devices=NCORES)

    KD = D // 128        # 8 contraction chunks over d
    NJ = N // 128        # 16 key blocks (j)
    CH = 512             # i-chunk width for stages B/C
    NCH = N // CH        # 4 chunks
    NSUB = CH // 128     # 4 row-subblocks per chunk

    xt_d = nc.dram_tensor("xt", [128, KD * N], H, kind="ExternalInput").ap()
    xn_d = nc.dram_tensor("xn", [N, D], H, kind="ExternalInput").ap()
    m4_d = nc.dram_tensor("m4", [128, KD * KD * 128], H, kind="ExternalInput").ap()
    vb_d = nc.dram_tensor("vb", [128, NJ], F, kind="ExternalInput").ap()
    ones_d = nc.dram_tensor("ones", [128, 8], H, kind="ExternalInput").ap()
    out_d = nc.dram_tensor("out", [N, D], H, kind="ExternalOutput").ap()

    with tile.TileContext(nc) as tc:
        with tc.tile_pool(name="xtp", bufs=1) as xtp, \
             tc.tile_pool(name="ytp", bufs=1) as ytp, \
             tc.tile_pool(name="xvp", bufs=1) as xvp, \
             tc.tile_pool(name="mwp", bufs=1) as mwp, \
             tc.tile_pool(name="misc", bufs=1) as misc, \
             tc.tile_pool(name="etp", bufs=1) as etp, \
             tc.tile_pool(name="esp", bufs=2) as esp, \
             tc.tile_pool(name="obp", bufs=2) as obp:

            xt = xtp.tile([128, KD * N], H, tag="xt", name="xt")
            xtr = xt.rearrange("p (k i) -> p k i", k=KD)
            xt_dr = xt_d.rearrange("p (k i) -> p k i", k=KD)
            yt = [ytp.tile([128, N], H, tag=f"yt{m}", name=f"yt{m}")
                  for m in range(KD)]
            # x natural strips, packed 4-per-tile so the load is 4 posts.
            xv4 = [xvp.tile([128, 4 * D], H, tag=f"xv{t}", name=f"xv{t}")
                   for t in range(NJ // 4)]
            xv = [xv4[j // 4][:, (j % 4) * D:(j % 4 + 1) * D] for j in range(NJ)]
            mw = [mwp.tile([128, KD * 128], H, tag=f"mw{m}", name=f"mw{m}")
                  for m in range(KD)]

            # ---- DMA issue order = prefetch priority ----
            # The descriptor-post rate on the sync engine (~0.65us/post) and
            # HBM bandwidth are both scarce during the first ~15us, so the
            # stage-A critical path (mw[0] + the n=0 xt sliver, 1.25MB) goes
            # first as two posts; bulk transfers are posted strictly after
            # the data stage A consumes early.
            nc.sync.dma_start(out=mw[0], in_=m4_d[:, 0:KD * 128])
            nc.sync.dma_start(out=xtr[:, :, 0:512], in_=xt_dr[:, :, 0:512])
            for m in range(1, KD):
                nc.sync.dma_start(out=mw[m],
                                  in_=m4_d[:, m * KD * 128:(m + 1) * KD * 128])
            for n in range(1, 4):
                nc.sync.dma_start(out=xtr[:, :, n * 512:(n + 1) * 512],
                                  in_=xt_dr[:, :, n * 512:(n + 1) * 512])
            vbt = misc.tile([128, NJ], F, tag="vb", name="vbt")
            nc.sync.dma_start(out=vbt, in_=vb_d)
            onesT = misc.tile([128, 8], H, tag="ones", name="onesT")
            nc.sync.dma_start(out=onesT, in_=ones_d)
            for t in range(NJ // 4):
                nc.sync.dma_start(
                    out=xv4[t].rearrange("p (jj d) -> p jj d", jj=4),
                    in_=xn_d[t * 512:(t + 1) * 512, :].rearrange(
                        "(jj p) d -> p jj d", p=128))

            # ---------------- stage A: y = x @ M ----------------
            with tc.tile_pool(name="psA", bufs=2, space="PSUM") as psA:
                # Warmup matmuls on zeroed SBUF: keep the PE busy through the
                # DMA lead-in so HAM reaches 8/8 before real work starts.
                warm = misc.tile([128, 640], H, tag="warm", name="warm")
                nc.any.memset(warm, 0.0)
                pw = psA.tile([128, 512], F, tag="w", name="psW", bufs=1)
                NWARM = 6
                for i in range(NWARM):
                    nc.tensor.matmul(pw, warm[:, 0:128], warm[:, 128:640],
                                     start=(i == 0), stop=(i == NWARM - 1))

                for n in range(4):
                    cols = slice(n * 512, (n + 1) * 512)
                    for m in range(KD):
                        pt = psA.tile([128, 512], F, tag=f"a{m % 2}",
                                      name=f"psA{m % 2}")
                        for k in range(KD):
                            nc.tensor.matmul(
                                pt, mw[m][:, k * 128:(k + 1) * 128],
                                xt[:, k * N + n * 512:k * N + (n + 1) * 512],
                                start=(k == 0), stop=(k == KD - 1))
                        nc.scalar.copy(yt[m][:, cols], pt)

            # ---------------- stages B + C, fused per i-chunk ----------------
            with tc.tile_pool(name="psB", bufs=2, space="PSUM") as psB, \
                 tc.tile_pool(name="psO", bufs=2, space="PSUM") as psO:
                et = [etp.tile([128, CH], H, tag=f"e{j}", name=f"e{j}")
                      for j in range(NJ)]
                for c in range(NCH):
                    ccols = slice(c * CH, (c + 1) * CH)
                    esum = esp.tile([128, CH], F, tag="esum", name="esum")
                    for j in range(NJ):
                        ps = psB.tile([128, CH], F, tag="sB", name="psB")
                        for k in range(KD):
                            nc.tensor.matmul(
                                ps,
                                xt[:, k * N + j * 128:k * N + j * 128 + 128],
                                yt[k][:, ccols],
                                start=(k == 0), stop=(k == KD - 1))
                        nc.scalar.activation(
                            et[j], ps, mybir.ActivationFunctionType.Exp,
                            bias=vbt[:, j:j + 1], scale=SCALE)
                        # running fp32 sum over j-strips for the denominator
                        if j == 0:
                            nc.vector.tensor_copy(esum, et[j])
                        else:
                            nc.vector.tensor_add(esum, esum, et[j])
                    esumR = esp.tile([128, CH], H, tag="esumR", name="esumR")
                    nc.vector.tensor_copy(esumR, esum)

                    for sub in range(NSUB):
                        p0 = psO.tile([128, 512], F, tag="c0", name="psO0")
                        p1 = psO.tile([128, 512], F, tag="c1", name="psO1")
                        for j in range(NJ):
                            lhs = et[j][:, sub * 128:(sub + 1) * 128]
                            nc.tensor.matmul(p0, lhs, xv[j][:, 0:512],
                                             start=(j == 0), stop=(j == NJ - 1))
                            nc.tensor.matmul(p1, lhs, xv[j][:, 512:1024],
                                             start=(j == 0), stop=(j == NJ - 1))
                        pd = psO.tile([128, 8], F, tag="cd", name="psOd")
                        nc.tensor.matmul(pd, esumR[:, sub * 128:(sub + 1) * 128],
                                         onesT, start=True, stop=True)
                        rden = obp.tile([128, 1], F, tag="rden", name="rden")
                        nc.vector.reciprocal(rden, pd[:, 0:1])
                        ob = obp.tile([128, D], H, tag="ob", name="ob")
                        # normalize halves on different engines (DVE + ACT) so
                        # they run in parallel; DMA each half as it completes.
                        row = c * CH + sub * 128
                        nc.vector.tensor_scalar_mul(ob[:, 0:512], p0, rden)
                        nc.sync.dma_start(out=out_d[row:row + 128, 0:512],
                                          in_=ob[:, 0:512])
                        nc.scalar.mul(ob[:, 512:1024], p1, rden)
                        nc.sync.dma_start(out=out_d[row:row + 128, 512:1024],
                                          in_=ob[:, 512:1024])

    nc.finalize()
    return nc


def _get_nc():
    global _NC
    if _NC is None:
        _NC = _build_nc()
    return _NC


def _prep_shared(W_qk, b_qk):
    W_qk = np.ascontiguousarray(W_qk, dtype=np.float32)
    b_qk = np.asarray(b_qk, dtype=np.float32)
    Wq, Wk = W_qk[:D], W_qk[D:]
    M = (Wk.T @ Wq).astype(np.float32)
    # m4[p, (m*8+k)*128+c] = M[k*128+p, m*128+c]
    m4 = np.ascontiguousarray(
        M.reshape(8, 128, 8, 128).transpose(1, 2, 0, 3).reshape(128, -1)
    ).astype(np.float16)
    wv = (Wq.T @ b_qk[D:]).astype(np.float32)      # v_j = x_j . wv
    ones = np.ones((128, 8), dtype=np.float16)
    return m4, wv, ones


def _host_inputs(x_b, m4, wv, ones):
    xt = np.ascontiguousarray(
        x_b.T.reshape(8, 128, N).transpose(1, 0, 2).reshape(128, -1)
    ).astype(np.float16)
    v = (x_b @ wv) * np.float32(SCALE)
    vb = np.ascontiguousarray(v.reshape(16, 128).T).astype(np.float32)
    return {
        "xt": xt,
        "xn": x_b.astype(np.float16),
        "m4": m4,
        "vb": vb,
        "ones": ones,
    }


def kernel(x: np.ndarray, W_qk: np.ndarray, b_qk: np.ndarray) -> np.ndarray:
    global LAST_RESULTS
    assert x.shape == (B, N, D), x.shape
    nc = _get_nc()

    x = np.ascontiguousarray(x, dtype=np.float32)
    m4, wv, ones = _prep_shared(W_qk, b_qk)
    in_maps = [_host_inputs(x[c], m4, wv, ones) for c in range(NCORES)]

    res = run_bass_kernel_spmd(nc, in_maps, core_ids=list(range(NCORES)))
    LAST_RESULTS = res
    out = np.stack([res.results[c]["out"] for c in range(NCORES)], axis=0)
    return out.astype(np.float32)


if __name__ == "__main__":
    rng = np.random.default_rng(0)
    x = rng.standard_normal((B, N, D), dtype=np.float32)
    limit = float(np.sqrt(6.0 / (D + 2 * D)))
    W = rng.uniform(-limit, limit, size=(2 * D, D)).astype(np.float32)
    b = np.zeros((2 * D,), dtype=np.float32)
    got = kernel(x, W, b)
    print("out", got.shape, got.dtype)


# revision 12
# speedup vs baseline: 1.0004x; 1.0004x over previous
"""Trainium2 Bass kernel for single-head attention (no V projection).

Reference computation (per batch b):
    qk   = x @ W_qk.T + b_qk          # [n, 2d]
    q, k = qk[:, :d], qk[:, d:]
    dots[i, j] = k_i . q_j / sqrt(d)
    attn = softmax(dots, axis=-1)
    out[i] = sum_j attn[i, j] * x[j]

Because there is no V projection, the q/k projections fold into a single
matrix: dots = x @ M @ x.T / sqrt(d) with M = Wk.T @ Wq precomputed on host
([d, d]).  That replaces the [n,d]x[d,2d] projection (8.6 GF) with
y = x @ M (4.3 GF) and keeps y on-chip (no DRAM spill), cutting tensor-engine
work from 25.8 GF to 21.5 GF per core.  The bias enters softmax only through
a per-key offset v_j = x_j . (Wq.T @ bk) (row-constant terms cancel), folded
into the exp as a per-partition ACT bias.

Sharding: data-parallel over batch b (8 batches -> 8 NeuronCores), no
collectives.  Per core:

  A:  yT[m,:] = M^T-chunks^T @ xT      (PE, fp16 operands, fp32 PSUM)
  B:  sT[j,i] = xT-chunk^T @ yT-chunk; E^T = exp(sT/32 + v_j)  (ACT)
  C:  out[i,:] = E^T(:,i)^T @ X in PSUM over j; softmax denominator from a
      DVE running sum of E^T strips + one tiny ones-matmul per 128-row block;
      normalize via per-partition reciprocal (DVE).

exp() is computed without max-subtraction: scores are ~N(0, 0.67) after the
1/sqrt(d) scale, so exp never overflows (max |s/32| ~ 5.4 on this data) and
softmax(x) == exp(x)/sum(exp(x)).

All matmul operands are fp16 (full PE rate; rel err ~6e-4 vs fp32 reference).

Host-side input layouts (computed in kernel()):
  m4 [128, 8*8*128]: m4[p, ((m*8)+k)*128+c] = M[k*128+p, m*128+c]
  xt [128, 8*2048]:  xt[p, k*2048+i] = x[i, k*128+p]
  xn [2048, 1024]:   x in fp16
  vb [128, 16]:      vb[p, j] = (x @ (Wq.T@bk))[j*128+p] / 32
"""
import sys

try:
    import concourse.bass as bass  # noqa: F401
except ImportError:  # pragma: no cover
    sys.path.insert(0, "/opt/trn_rl_repo")

import numpy as np
import concourse.bass as bass
import concourse.mybir as mybir
import concourse.tile as tile
from concourse import bacc
from concourse.bass_utils import run_bass_kernel_spmd
import concourse.bass_utils as _bu

# NOTE: unlike the fp32r version, we do NOT patch walrus to
# --enable-ldw-opt=true: fp16 weights take the Fast-Weight-Load path, which
# that optimization rejects (InstLdweights incompatible), and FWL's 2x faster
# weight loads matter more than eliding the duplicate LDWEIGHTS in stage C.

B, N, D = 8, 2048, 1024
NCORES = 8
SCALE = 1.0 / np.sqrt(D)  # 1/32

_NC = None
LAST_RESULTS = None


def _build_nc():
    H = mybir.dt.float16
    F = mybir.dt.float32
    nc = bacc.Bacc("TRN2", target_bir_lowering=False, debug=False, num_devices=NCORES)

    KD = D // 128        # 8 contraction chunks over d
    NJ = N // 128        # 16 key blocks (j)
    CH = 512             # i-chunk width for stages B/C
    NCH = N // CH        # 4 chunks
    NSUB = CH // 128     # 4 row-subblocks per chunk

    xt_d = nc.dram_tensor("xt", [128, KD * N], H, kind="ExternalInput").ap()
    xn_d = nc.dram_tensor("xn", [N, D], H, kind="ExternalInput").ap()
    m4_d = nc.dram_tensor("m4", [128, KD * KD * 128], H, kind="ExternalInput").ap()
    vb_d = nc.dram_tensor("vb", [128, NJ], F, kind="ExternalInput").ap()
    ones_d = nc.dram_tensor("ones", [128, 8], H, kind="ExternalInput").ap()
    out_d = nc.dram_tensor("out", [N, D], H, kind="ExternalOutput").ap()

    with tile.TileContext(nc) as tc:
        with tc.tile_pool(name="xtp", bufs=1) as xtp, \
             tc.tile_pool(name="ytp", bufs=1) as ytp, \
             tc.tile_pool(name="xvp", bufs=1) as xvp, \
             tc.tile_pool(name="mwp", bufs=1) as mwp, \
             tc.tile_pool(name="misc", bufs=1) as misc, \
             tc.tile_pool(name="etp", bufs=1) as etp, \
             tc.tile_pool(name="esp", bufs=2) as esp, \
             tc.tile_pool(name="obp", bufs=2) as obp:

            # xt layout [p, n, k, c]: the n=0 block (everything stage A's first
            # column chunk needs) is one contiguous 8KB-per-partition DMA.
            xt = xtp.tile([128, KD * N], H, tag="xt", name="xt")

            def xt_sl(k, col0, w):
                # xT[k*128+p, col0:col0+w]; requires col0 % 512 + w <= 512
                n, off = col0 // 512, col0 % 512
                base = (n * KD + k) * 512 + off
                return xt[:, base:base + w]

            yt = [ytp.tile([128, N], H, tag=f"yt{m}", name=f"yt{m}")
                  for m in range(KD)]
            # x natural strips, packed 4-per-tile so the load is 4 posts.
            xv4 = [xvp.tile([128, 4 * D], H, tag=f"xv{t}", name=f"xv{t}")
                   for t in range(NJ // 4)]
            xv = [xv4[j // 4][:, (j % 4) * D:(j % 4 + 1) * D] for j in range(NJ)]
            mw = [mwp.tile([128, KD * 128], H, tag=f"mw{m}", name=f"mw{m}")
                  for m in range(KD)]

            # ---- DMA issue order = prefetch priority ----
            # The descriptor-post rate on the sync engine (~0.65us/post) and
            # HBM bandwidth are both scarce during the first ~15us, so the
            # stage-A critical path (mw[0] + the n=0 xt sliver, 1.25MB) goes
            # first as two posts; bulk transfers are posted strictly after
            # the data stage A consumes early.
            nc.sync.dma_start(out=mw[0], in_=m4_d[:, 0:KD * 128])
            nc.sync.dma_start(out=xt[:, 0:KD * 512], in_=xt_d[:, 0:KD * 512])
            for m in range(1, KD):
                nc.sync.dma_start(out=mw[m],
                                  in_=m4_d[:, m * KD * 128:(m + 1) * KD * 128])
            nc.sync.dma_start(out=xt[:, KD * 512:4 * KD * 512],
                              in_=xt_d[:, KD * 512:4 * KD * 512])
            vbt = misc.tile([128, NJ], F, tag="vb", name="vbt")
            nc.sync.dma_start(out=vbt, in_=vb_d)
            onesT = misc.tile([128, 8], H, tag="ones", name="onesT")
            nc.sync.dma_start(out=onesT, in_=ones_d)
            for t in range(NJ // 4):
                nc.sync.dma_start(
                    out=xv4[t].rearrange("p (jj d) -> p jj d", jj=4),
                    in_=xn_d[t * 512:(t + 1) * 512, :].rearrange(
                        "(jj p) d -> p jj d", p=128))

            # ---------------- stage A: y = x @ M ----------------
            with tc.tile_pool(name="psA", bufs=2, space="PSUM") as psA:
                # Warmup matmuls on zeroed SBUF: keep the PE busy through the
                # DMA lead-in so HAM reaches 8/8 before real work starts.
                warm = misc.tile([128, 640], H, tag="warm", name="warm")
                nc.any.memset(warm, 0.0)
                pw = psA.tile([128, 512], F, tag="w", name="psW", bufs=1)
                NWARM = 14
                for i in range(NWARM):
                    nc.tensor.matmul(pw, warm[:, 0:128], warm[:, 128:640],
                                     start=(i == 0), stop=(i == NWARM - 1))

                for n in range(4):
                    cols = slice(n * 512, (n + 1) * 512)
                    for m in range(KD):
                        pt = psA.tile([128, 512], F, tag=f"a{m % 2}",
                                      name=f"psA{m % 2}")
                        for k in range(KD):
                            nc.tensor.matmul(
                                pt, mw[m][:, k * 128:(k + 1) * 128],
                                xt_sl(k, n * 512, 512),
                                start=(k == 0), stop=(k == KD - 1))
                        nc.scalar.copy(yt[m][:, cols], pt)

            # ---------------- stages B + C, fused per i-chunk ----------------
            with tc.tile_pool(name="psB", bufs=2, space="PSUM") as psB, \
                 tc.tile_pool(name="psO", bufs=2, space="PSUM") as psO:
                et = [etp.tile([128, CH], H, tag=f"e{j}", name=f"e{j}")
                      for j in range(NJ)]
                for c in range(NCH):
                    ccols = slice(c * CH, (c + 1) * CH)
                    esum = esp.tile([128, CH], F, tag="esum", name="esum")
                    for j in range(NJ):
                        ps = psB.tile([128, CH], F, tag="sB", name="psB")
                        for k in range(KD):
                            nc.tensor.matmul(
                                ps, xt_sl(k, j * 128, 128), yt[k][:, ccols],
                                start=(k == 0), stop=(k == KD - 1))
                        nc.scalar.activation(
                            et[j], ps, mybir.ActivationFunctionType.Exp,
                            bias=vbt[:, j:j + 1], scale=SCALE)
                        # running fp32 sum over j-strips for the denominator
                        if j == 0:
                            nc.vector.tensor_copy(esum, et[j])
                        else:
                            nc.vector.tensor_add(esum, esum, et[j])
                    esumR = esp.tile([128, CH], H, tag="esumR", name="esumR")
                    nc.vector.tensor_copy(esumR, esum)

                    for sub in range(NSUB):
                        p0 = psO.tile([128, 512], F, tag="c0", name="psO0")
                        p1 = psO.tile([128, 512], F, tag="c1", name="psO1")
                        for j in range(NJ):
                            lhs = et[j][:, sub * 128:(sub + 1) * 128]
                            nc.tensor.matmul(p0, lhs, xv[j][:, 0:512],
                                             start=(j == 0), stop=(j == NJ - 1))
                            nc.tensor.matmul(p1, lhs, xv[j][:, 512:1024],
                                             start=(j == 0), stop=(j == NJ - 1))
                        pd = psO.tile([128, 8], F, tag="cd", name="psOd")
                        nc.tensor.matmul(pd, esumR[:, sub * 128:(sub + 1) * 128],
                                         onesT, start=True, stop=True)
                        rden = obp.tile([128, 1], F, tag="rden", name="rden")
                        nc.vector.reciprocal(rden, pd[:, 0:1])
                        ob = obp.tile([128, D], H, tag="ob", name="ob")
                        # normalize halves on different engines (DVE + ACT) so
                        # they run in parallel; DMA each half as it completes.
                        row = c * CH + sub * 128
                        nc.vector.tensor_scalar_mul(ob[:, 0:512], p0, rden)
                        nc.sync.dma_start(out=out_d[row:row + 128, 0:512],
                                          in_=ob[:, 0:512])
                        nc.scalar.mul(ob[:, 512:1024], p1, rden)
                        nc.sync.dma_start(out=out_d[row:row + 128, 512:1024],
                                          in_=ob[:, 512:1024])

    nc.finalize()
    return nc


def _get_nc():
    global _NC
    if _NC is None:
        _NC = _build_nc()
    return _NC


def _prep_shared(W_qk, b_qk):
    W_qk = np.ascontiguousarray(W_qk, dtype=np.float32)
    b_qk = np.asarray(b_qk, dtype=np.float32)
    Wq, Wk = W_qk[:D], W_qk[D:]
    M = (Wk.T @ Wq).astype(np.float32)
    # m4[p, (m*8+k)*128+c] = M[k*128+p, m*128+c]
    m4 = np.ascontiguousarray(
        M.reshape(8, 128, 8, 128).transpose(1, 2, 0, 3).reshape(128, -1)
    ).astype(np.float16)
    wv = (Wq.T @ b_qk[D:]).astype(np.float32)      # v_j = x_j . wv
    ones = np.ones((128, 8), dtype=np.float16)
    return m4, wv, ones


def _host_inputs(x_b, m4, wv, ones):
    # xt[p, ((n*8)+k)*512+c] = x[n*512+c, k*128+p]
    xt = np.ascontiguousarray(
        x_b.T.reshape(8, 128, 4, 512).transpose(1, 2, 0, 3).reshape(128, -1)
    ).astype(np.float16)
    v = (x_b @ wv) * np.float32(SCALE)
    vb = np.ascontiguousarray(v.reshape(16, 128).T).astype(np.float32)
    return {
        "xt": xt,
        "xn": x_b.astype(np.float16),
        "m4": m4,
        "vb": vb,
        "ones": ones,
    }


def kernel(x: np.ndarray, W_qk: np.ndarray, b_qk: np.ndarray) -> np.ndarray:
    global LAST_RESULTS
    assert x.shape == (B, N, D), x.shape
    nc = _get_nc()

    x = np.ascontiguousarray(x, dtype=np.float32)
    m4, wv, ones = _prep_shared(W_qk, b_qk)
    in_maps = [_host_inputs(x[c], m4, wv, ones) for c in range(NCORES)]

    res = run_bass_kernel_spmd(nc, in_maps, core_ids=list(range(NCORES)))
    LAST_RESULTS = res
    out = np.stack([res.results[c]["out"] for c in range(NCORES)], axis=0)
    return out.astype(np.float32)


if __name__ == "__main__":
    rng = np.random.default_rng(0)
    x = rng.standard_normal((B, N, D), dtype=np.float32)
    limit = float(np.sqrt(6.0 / (D + 2 * D)))
    W = rng.uniform(-limit, limit, size=(2 * D, D)).astype(np.float32)
    b = np.zeros((2 * D,), dtype=np.float32)
    got = kernel(x, W, b)
    print("out", got.shape, got.dtype)
